# revision 8
# baseline (speedup 1.0000x reference)
"""Trainium2 Bass kernel for nn_LinformerProjectionEntireOutImg.

Math: the reference's softmax is over a constant tensor -> uniform 1/64, so
the whole net collapses to a linear pipeline:
  T[b,q,i,j]  = sum_p cp[b, p*128+q, i, :] @ wc[p*128+q, :, j]   (q = n mod 128)
  S[b, r]     = T.reshape(B, 8192),  r = q*64 + i*8 + j
  P2[b,e]     = S @ E_proj.reshape(8192, 256)
  v[b,k]      = (P2[b,k]+P2[b,64+k]+P2[b,128+k]+P2[b,192+k])/64 + rel[k]
  out[b,o,i,j]= sum_m v[b, i*8+m] * w_next[o, m, j]
Sharding: core c owns capsule groups q in [16c, 16c+16) (== heads 4c..4c+4),
batch unsharded. Each core reads a disjoint 1/8 of current_pose/w_current and
1/8 of E_proj. The pipeline is linear, so each core emits its partial output
(core 0 carries the rel_embedd affine term) and the unshard is a sum.

Precision plan (HBM traffic is the bottleneck; 358 GB/s/core):
  stage 1 operands in fp8e4 (A raw randn; W pre-scaled x64 on host so its
  0.02*randn values sit in e4m3's normal range), fp32 PSUM accumulation.
  The 4-way hid fold (256->64) plus the 1/64 softmax mean plus the 1/64
  W-scale compensation are all folded into E on the host -> E shrinks to
  [128,512] bf16.  Stage 2/3 run in bf16, output in bf16 (host sums cores
  in fp32).  Measured end-to-end rel err ~3e-3 vs the 2e-2 gate.
"""

import os

import numpy as np

_STATE: dict = {}

B, OUT_N, POSE = 32, 64, 64
NCORES = 8

# p-chunk boundaries for the streamed stage-1 operand. DMA queues issue
# ~45-75 packets/us and packets never span partition rows, so chunks must
# keep rows fat (10 p = 3840 B/row, one near-4KB packet per row); a small
# first chunk lets the PE start early.
P_BOUNDS = [0, 4, 14, 24, 34, 44, 54, 64]
WARMUP_MM = 7  # dummy matmuls to start the PE DVFS ramp during the DMA lead-in


def _build_nc():
    import concourse.mybir as mybir
    from concourse import bacc
    from concourse.tile import TileContext

    f32 = mybir.dt.float32
    bf16 = mybir.dt.bfloat16
    f8 = mybir.dt.float8e4
    nc = bacc.Bacc()
    # AW pack: per p, 256 fp8 cols of A ((i,b) major) then 128 fp8 cols of
    # block-diag W -> 384 B/partition/p.
    AW = nc.dram_tensor("aw_pack", [128, 64 * 384], f8, kind="ExternalInput")
    E = nc.dram_tensor("e_pack", [128, 512], bf16, kind="ExternalInput")
    REL = nc.dram_tensor("rel32", [32, 64], f32, kind="ExternalInput")
    WN = nc.dram_tensor("wn_pack", [8, 512], bf16, kind="ExternalInput")
    IDT = nc.dram_tensor("ident32", [32, 32], bf16, kind="ExternalInput")
    OUT = nc.dram_tensor("out", [128, 1024], bf16, kind="ExternalOutput")

    with TileContext(nc) as tc:
        with (
            tc.tile_pool(name="apool", bufs=len(P_BOUNDS) - 1) as apool,
            tc.tile_pool(name="cpool", bufs=1) as cpool,
            tc.tile_pool(name="spool", bufs=1) as spool,
            tc.tile_pool(name="pp", bufs=1, space="PSUM") as pp,
            tc.tile_pool(name="pp3", bufs=2, space="PSUM") as pp3,
        ):
            # PE warmup: dummy matmuls on a zeroed scratch tile keep the PE
            # busy during the DMA lead-in so the DVFS ramp starts early.
            zt = cpool.tile([128, 512], f8, tag="zt")
            nc.gpsimd.memset(zt[:], 0)
            warm_ps = pp.tile([128, 512], f32, tag="warm")
            for _ in range(WARMUP_MM):
                nc.tensor.matmul(
                    warm_ps[:], zt[:, 0:128], zt[:], start=True, stop=True
                )

            # AW chunk DMAs, alternating between the two HWDGE queues.
            awts = []
            for ci in range(len(P_BOUNDS) - 1):
                w = (P_BOUNDS[ci + 1] - P_BOUNDS[ci]) * 384
                awt = apool.tile([128, w], f8, tag="aw")
                eng = (nc.sync, nc.scalar)[ci % 2]
                eng.dma_start(
                    out=awt[:],
                    in_=AW[:, P_BOUNDS[ci] * 384 : P_BOUNDS[ci + 1] * 384],
                )
                awts.append(awt)
            # params land after the A stream: they're only needed at stage 2/3
            et = cpool.tile([128, 512], bf16, tag="e")
            nc.scalar.dma_start(out=et[:], in_=E[:])
            relt = cpool.tile([32, 64], f32, tag="rel")
            nc.sync.dma_start(out=relt[:], in_=REL[:])
            idt = cpool.tile([32, 32], bf16, tag="idt")
            nc.scalar.dma_start(out=idt[:], in_=IDT[:])
            wnt = cpool.tile([8, 512], bf16, tag="wn")
            nc.sync.dma_start(out=wnt[:], in_=WN[:])

            # stage 1: T[(q,j),(i,b)] = sum_p Wblk_p.T @ A_p  (block-diag over q)
            # Two interleaved accumulation chains (even/odd p) in separate
            # PSUM banks so per-matmul ordering waits don't serialize the PE.
            o_ps0 = pp.tile([128, 256], f32, tag="o_ps0")
            o_ps1 = pp.tile([128, 256], f32, tag="o_ps1")
            for ci in range(len(P_BOUNDS) - 1):
                awt = awts[ci]
                for t in range(P_BOUNDS[ci + 1] - P_BOUNDS[ci]):
                    p = P_BOUNDS[ci] + t
                    tgt = o_ps0 if p % 2 == 0 else o_ps1
                    nc.tensor.matmul(
                        tgt[:],
                        awt[:, t * 384 + 256 : (t + 1) * 384],
                        awt[:, t * 384 : t * 384 + 256],
                        start=(p < 2),
                        stop=(p >= 62),
                    )
            o_half = spool.tile([128, 256], f32, tag="ohalf")
            nc.vector.tensor_copy(o_half[:], o_ps0[:])
            o_sb = spool.tile([128, 256], bf16, tag="osb")
            nc.vector.tensor_add(o_sb[:], o_half[:], o_ps1[:])

            # stage 2: v[b,k] += O[:, i-cols].T @ Ef_i  (accumulate over i)
            # Ef has the 4-way hid fold, the softmax 1/64 and the W x64
            # compensation baked in.
            p2_ps = pp.tile([32, 64], f32, tag="p2_ps")
            for i in range(8):
                nc.tensor.matmul(
                    p2_ps[:],
                    o_sb[:, i * 32 : (i + 1) * 32],
                    et[:, i * 64 : (i + 1) * 64],
                    start=(i == 0),
                    stop=(i == 7),
                )

            # add rel (zeros on cores 1..7), cast to bf16
            vs = spool.tile([32, 64], bf16, tag="vs")
            nc.vector.tensor_add(vs[:], p2_ps[:], relt[:])

            # transpose v slices: vt[m, i*32+b] = v[b, i*8+m] (partition base 0)
            vt_ps = pp.tile([8, 256], bf16, tag="vt_ps")
            for i in range(8):
                nc.tensor.transpose(
                    vt_ps[:, i * 32 : (i + 1) * 32],
                    vs[:, i * 8 : (i + 1) * 8],
                    idt[:],
                )
            vt_sb = spool.tile([8, 256], bf16, tag="vt")
            nc.vector.tensor_copy(vt_sb[:], vt_ps[:])

            # stage 3: out_h[(i4,b),(o,j)] = vt[:, h-cols].T @ wn[m,(o,j)]
            # Both halves land in one [128,1024] tile (2KB rows) and go out
            # as two partition-split DMAs so both queues push fat packets.
            o3_sb = spool.tile([128, 1024], bf16, tag="o3sb")
            for h in range(2):
                o3 = pp3.tile([128, 512], f32, tag="o3")
                nc.tensor.matmul(
                    o3[:],
                    vt_sb[:, h * 128 : (h + 1) * 128],
                    wnt[:],
                    start=True,
                    stop=True,
                )
                if h == 0:
                    nc.vector.tensor_copy(o3_sb[:, 0:512], o3[:])
                else:
                    nc.scalar.copy(o3_sb[:, 512:1024], o3[:])
            nc.sync.dma_start(out=OUT[0:64, :], in_=o3_sb[0:64, :])
            nc.scalar.dma_start(out=OUT[64:128, :], in_=o3_sb[64:128, :])
    nc.finalize()
    return nc


def _prepack(current_pose, w_current, w_next, E_proj, rel_embedd):
    import ml_dtypes

    f8 = ml_dtypes.float8_e4m3fn
    cp = np.ascontiguousarray(current_pose, dtype=np.float32)
    wc = np.ascontiguousarray(w_current, dtype=np.float32).reshape(64, 8, 16, 8, 8)
    # A_all[c, p, (q,m), (i,b)]
    cp6 = cp.reshape(B, 64, 8, 16, 8, 8)  # (b, p, c, q, i, m)
    a_all = np.ascontiguousarray(cp6.transpose(2, 1, 3, 5, 4, 0), dtype=f8).reshape(
        8, 64, 128, 256
    )
    # Wblk_all[c, p, (q,m), (q',j)] block-diagonal; x64 so 0.02*randn values
    # land in e4m3's normal range (compensated in E)
    w_all = np.zeros((8, 64, 16, 8, 16, 8), dtype=f8)
    wc_t = np.ascontiguousarray(wc.transpose(1, 0, 2, 3, 4)) * 64.0  # (c,p,q,m,j)
    wc_t8 = wc_t.astype(f8)
    for q in range(16):
        w_all[:, :, q, :, q, :] = wc_t8[:, :, q]
    w_all = w_all.reshape(8, 64, 128, 128)
    aw_all = np.concatenate([a_all, w_all], axis=-1)  # (c, p, 128, 384)
    # -> (c, part, (p, x)) flat columns
    aw_all = np.ascontiguousarray(aw_all.transpose(0, 2, 1, 3)).reshape(
        8, 128, 64 * 384
    )
    # Ef[c, (q,j), (i,k)]: hid fold (sum over h4), /64 softmax mean, /64 W-scale
    e6 = (np.asarray(E_proj, dtype=np.float32) / 4096.0).reshape(
        8, 4, 4, 8, 8, 4, 64
    )  # (c, qh, q4, i, j, h4, k)
    ef = e6.sum(axis=5)  # (c, qh, q4, i, j, k)
    ef = np.ascontiguousarray(
        ef.transpose(0, 1, 2, 4, 3, 5), dtype=ml_dtypes.bfloat16
    ).reshape(8, 128, 512)  # (c, (qh,q4,j), (i,k))
    # rel tile: only core 0 carries the affine term
    rel_all = np.zeros((8, 32, 64), dtype=np.float32)
    rel_all[0] = np.broadcast_to(
        np.asarray(rel_embedd, dtype=np.float32).reshape(1, 64), (32, 64)
    )
    wn_pack = np.ascontiguousarray(
        np.asarray(w_next, dtype=np.float32).transpose(1, 0, 2).reshape(8, 512),
        dtype=ml_dtypes.bfloat16,
    )
    ident = np.eye(32, dtype=ml_dtypes.bfloat16)
    in_maps = []
    for c in range(NCORES):
        in_maps.append(
            {
                "aw_pack": aw_all[c],
                "e_pack": ef[c],
                "rel32": rel_all[c],
                "wn_pack": wn_pack,
                "ident32": ident,
            }
        )
    return in_maps


def kernel(current_pose, w_current, w_next, E_proj, rel_embedd):
    from concourse import bass_utils

    if "nc" not in _STATE:
        _STATE["nc"] = _build_nc()
    nc = _STATE["nc"]
    in_maps = _prepack(current_pose, w_current, w_next, E_proj, rel_embedd)
    trace = os.environ.get("KERNEL_TRACE") == "1"
    res = bass_utils.run_bass_kernel_spmd(
        nc, in_maps, core_ids=list(range(NCORES)), trace=trace
    )
    _STATE["last_result"] = res
    acc = np.zeros((128, 1024), dtype=np.float32)
    for c in range(NCORES):
        acc += np.asarray(res.results[c]["out"], dtype=np.float32)
    # [(i4, b), (h, o, j)] -> (b, o, h*4+i4, j)
    out = (
        acc.reshape(4, 32, 2, 64, 8)
        .transpose(1, 3, 2, 0, 4)
        .reshape(B, OUT_N, POSE)
    )
    return np.ascontiguousarray(out[:, None, :, :])


# revision 11
# speedup vs baseline: 1.2379x; 1.2379x over previous
"""Trainium2 Bass kernel for nn_LinformerProjectionEntireOutImg.

Math: the reference's softmax is over a constant tensor -> uniform 1/64, so
the whole net collapses to a linear pipeline:
  T[b,q,i,j]  = sum_p cp[b, p*128+q, i, :] @ wc[p*128+q, :, j]   (q = n mod 128)
  S[b, r]     = T.reshape(B, 8192),  r = q*64 + i*8 + j
  P2[b,e]     = S @ E_proj.reshape(8192, 256)
  v[b,k]      = (P2[b,k]+P2[b,64+k]+P2[b,128+k]+P2[b,192+k])/64 + rel[k]
  out[b,o,i,j]= sum_m v[b, i*8+m] * w_next[o, m, j]
Sharding: core c owns capsule groups q in [16c, 16c+16) (== heads 4c..4c+4),
batch unsharded. Each core reads a disjoint 1/8 of current_pose/w_current and
1/8 of E_proj. The pipeline is linear, so each core emits its partial output
(core 0 carries the rel_embedd affine term) and the unshard is a sum.

Precision plan (HBM traffic is the bottleneck; 358 GB/s/core):
  stage 1 operands in fp8e4 (A raw randn; W pre-scaled x64 on host so its
  0.02*randn values sit in e4m3's normal range), fp32 PSUM accumulation.
  The 4-way hid fold (256->64) plus the 1/64 softmax mean plus the 1/64
  W-scale compensation are all folded into E on the host -> E shrinks to
  [128,512] bf16.  Stage 2/3 run in bf16, output in bf16 (host sums cores
  in fp32).  Measured end-to-end rel err ~3e-3 vs the 2e-2 gate.
"""

import os

import numpy as np

_STATE: dict = {}

B, OUT_N, POSE = 32, 64, 64
NCORES = 8

# p-chunk boundaries for the streamed stage-1 operand. DMA queues issue
# ~45-75 packets/us and packets never span partition rows, so chunks must
# keep rows fat (10 p = 3840 B/row, one near-4KB packet per row); a small
# first chunk lets the PE start early. All chunks even-sized (DoubleRow
# consumes p in pairs).
P_BOUNDS = [0, 4, 14, 24, 34, 44, 54, 64]


def _build_nc():
    import concourse.mybir as mybir
    from concourse import bacc
    from concourse.tile import TileContext

    f32 = mybir.dt.float32
    bf16 = mybir.dt.bfloat16
    f8 = mybir.dt.float8e4
    nc = bacc.Bacc()
    # AW pack: per p, 256 fp8 cols of A ((i,b) major) then 128 fp8 cols of
    # block-diag W -> 384 B/partition/p.
    AW = nc.dram_tensor("aw_pack", [128, 64 * 384], f8, kind="ExternalInput")
    E = nc.dram_tensor("e_pack", [128, 512], bf16, kind="ExternalInput")
    REL = nc.dram_tensor("rel32", [32, 64], f32, kind="ExternalInput")
    WN = nc.dram_tensor("wn_pack", [8, 512], bf16, kind="ExternalInput")
    IDT = nc.dram_tensor("ident32", [32, 32], bf16, kind="ExternalInput")
    OUT = nc.dram_tensor("out", [128, 1024], bf16, kind="ExternalOutput")

    with TileContext(nc) as tc:
        with (
            tc.tile_pool(name="apool", bufs=len(P_BOUNDS) - 1) as apool,
            tc.tile_pool(name="cpool", bufs=1) as cpool,
            tc.tile_pool(name="spool", bufs=1) as spool,
            tc.tile_pool(name="pp", bufs=1, space="PSUM") as pp,
            tc.tile_pool(name="pp3", bufs=2, space="PSUM") as pp3,
        ):
            # AW chunk DMAs, alternating between the two HWDGE queues.
            # Tiles are [128, pairs, 2, 384] so DoubleRow can slice two
            # consecutive p per matmul ([:, pr, :, 0:256] A / [:, pr, :,
            # 256:384] W).
            awts = []
            for ci in range(len(P_BOUNDS) - 1):
                npair = (P_BOUNDS[ci + 1] - P_BOUNDS[ci]) // 2
                awt = apool.tile([128, npair, 2, 384], f8, tag="aw")
                eng = (nc.sync, nc.scalar)[ci % 2]
                eng.dma_start(
                    out=awt[:],
                    in_=AW[:, P_BOUNDS[ci] * 384 : P_BOUNDS[ci + 1] * 384],
                )
                awts.append(awt)
            # params are only needed at stage 2/3: deprioritize so the Tile
            # scheduler queues them behind every AW chunk.
            prio = tc.cur_priority
            tc.cur_priority = 1 << 20
            et = cpool.tile([128, 512], bf16, tag="e")
            nc.scalar.dma_start(out=et[:], in_=E[:])
            relt = cpool.tile([32, 64], f32, tag="rel")
            nc.sync.dma_start(out=relt[:], in_=REL[:])
            idt = cpool.tile([32, 32], bf16, tag="idt")
            nc.scalar.dma_start(out=idt[:], in_=IDT[:])
            wnt = cpool.tile([8, 512], bf16, tag="wn")
            nc.sync.dma_start(out=wnt[:], in_=WN[:])
            tc.cur_priority = prio

            # stage 1: T[(q,j),(i,b)] = sum_p Wblk_p.T @ A_p  (block-diag over
            # q), two p per DoubleRow matmul. Two interleaved accumulation
            # chains (even/odd pairs) in separate PSUM banks so per-matmul
            # ordering waits don't serialize the PE.
            o_ps0 = pp.tile([128, 256], f32, tag="o_ps0")
            o_ps1 = pp.tile([128, 256], f32, tag="o_ps1")
            import concourse.mybir as _mb

            dr = _mb.MatmulPerfMode.DoubleRow
            pr_base = 0
            for ci in range(len(P_BOUNDS) - 1):
                awt = awts[ci]
                for t in range((P_BOUNDS[ci + 1] - P_BOUNDS[ci]) // 2):
                    pr = pr_base + t
                    tgt = o_ps0 if pr % 2 == 0 else o_ps1
                    nc.tensor.matmul(
                        tgt[:],
                        awt[:, t, :, 256:384],
                        awt[:, t, :, 0:256],
                        start=(pr < 2),
                        stop=(pr >= 30),
                        perf_mode=dr,
                    )
                pr_base += (P_BOUNDS[ci + 1] - P_BOUNDS[ci]) // 2
            o_half = spool.tile([128, 256], f32, tag="ohalf")
            nc.scalar.copy(o_half[:], o_ps0[:])
            o_sb = spool.tile([128, 256], bf16, tag="osb")
            nc.vector.tensor_add(o_sb[:], o_half[:], o_ps1[:])

            # stage 2: v[b,k] += O[:, i-cols].T @ Ef_i  (accumulate over i)
            # Ef has the 4-way hid fold, the softmax 1/64 and the W x64
            # compensation baked in.
            p2_ps = pp.tile([32, 64], f32, tag="p2_ps")
            for i in range(8):
                nc.tensor.matmul(
                    p2_ps[:],
                    o_sb[:, i * 32 : (i + 1) * 32],
                    et[:, i * 64 : (i + 1) * 64],
                    start=(i == 0),
                    stop=(i == 7),
                )

            # add rel (zeros on cores 1..7), cast to bf16
            vs = spool.tile([32, 64], bf16, tag="vs")
            nc.vector.tensor_add(vs[:], p2_ps[:], relt[:])

            # transpose v slices: vt[m, i*32+b] = v[b, i*8+m] (partition base 0)
            vt_ps = pp.tile([8, 256], bf16, tag="vt_ps")
            for i in range(8):
                nc.tensor.transpose(
                    vt_ps[:, i * 32 : (i + 1) * 32],
                    vs[:, i * 8 : (i + 1) * 8],
                    idt[:],
                )
            vt_sb = spool.tile([8, 256], bf16, tag="vt")
            nc.vector.tensor_copy(vt_sb[:], vt_ps[:])

            # stage 3: out_h[(i4,b),(o,j)] = vt[:, h-cols].T @ wn[m,(o,j)]
            # Both halves land in one [128,1024] tile (2KB rows) and go out
            # as two partition-split DMAs so both queues push fat packets.
            o3_sb = spool.tile([128, 1024], bf16, tag="o3sb")
            for h in range(2):
                o3 = pp3.tile([128, 512], f32, tag="o3")
                nc.tensor.matmul(
                    o3[:],
                    vt_sb[:, h * 128 : (h + 1) * 128],
                    wnt[:],
                    start=True,
                    stop=True,
                )
                if h == 0:
                    nc.vector.tensor_copy(o3_sb[:, 0:512], o3[:])
                else:
                    nc.scalar.copy(o3_sb[:, 512:1024], o3[:])
            nc.sync.dma_start(out=OUT[0:64, :], in_=o3_sb[0:64, :])
            nc.scalar.dma_start(out=OUT[64:128, :], in_=o3_sb[64:128, :])
    nc.finalize()
    return nc


def _prepack(current_pose, w_current, w_next, E_proj, rel_embedd):
    import ml_dtypes

    f8 = ml_dtypes.float8_e4m3fn
    cp = np.ascontiguousarray(current_pose, dtype=np.float32)
    wc = np.ascontiguousarray(w_current, dtype=np.float32).reshape(64, 8, 16, 8, 8)
    # A_all[c, p, (q,m), (i,b)]
    cp6 = cp.reshape(B, 64, 8, 16, 8, 8)  # (b, p, c, q, i, m)
    a_all = np.ascontiguousarray(cp6.transpose(2, 1, 3, 5, 4, 0), dtype=f8).reshape(
        8, 64, 128, 256
    )
    # Wblk_all[c, p, (q,m), (q',j)] block-diagonal; x64 so 0.02*randn values
    # land in e4m3's normal range (compensated in E)
    w_all = np.zeros((8, 64, 16, 8, 16, 8), dtype=f8)
    wc_t = np.ascontiguousarray(wc.transpose(1, 0, 2, 3, 4)) * 64.0  # (c,p,q,m,j)
    wc_t8 = wc_t.astype(f8)
    for q in range(16):
        w_all[:, :, q, :, q, :] = wc_t8[:, :, q]
    w_all = w_all.reshape(8, 64, 128, 128)
    aw_all = np.concatenate([a_all, w_all], axis=-1)  # (c, p, 128, 384)
    # -> (c, part, (p, x)) flat columns
    aw_all = np.ascontiguousarray(aw_all.transpose(0, 2, 1, 3)).reshape(
        8, 128, 64 * 384
    )
    # Ef[c, (q,j), (i,k)]: hid fold (sum over h4), /64 softmax mean, /64 W-scale
    e6 = (np.asarray(E_proj, dtype=np.float32) / 4096.0).reshape(
        8, 4, 4, 8, 8, 4, 64
    )  # (c, qh, q4, i, j, h4, k)
    ef = e6.sum(axis=5)  # (c, qh, q4, i, j, k)
    ef = np.ascontiguousarray(
        ef.transpose(0, 1, 2, 4, 3, 5), dtype=ml_dtypes.bfloat16
    ).reshape(8, 128, 512)  # (c, (qh,q4,j), (i,k))
    # rel tile: only core 0 carries the affine term
    rel_all = np.zeros((8, 32, 64), dtype=np.float32)
    rel_all[0] = np.broadcast_to(
        np.asarray(rel_embedd, dtype=np.float32).reshape(1, 64), (32, 64)
    )
    wn_pack = np.ascontiguousarray(
        np.asarray(w_next, dtype=np.float32).transpose(1, 0, 2).reshape(8, 512),
        dtype=ml_dtypes.bfloat16,
    )
    ident = np.eye(32, dtype=ml_dtypes.bfloat16)
    in_maps = []
    for c in range(NCORES):
        in_maps.append(
            {
                "aw_pack": aw_all[c],
                "e_pack": ef[c],
                "rel32": rel_all[c],
                "wn_pack": wn_pack,
                "ident32": ident,
            }
        )
    return in_maps


def kernel(current_pose, w_current, w_next, E_proj, rel_embedd):
    from concourse import bass_utils

    if "nc" not in _STATE:
        _STATE["nc"] = _build_nc()
    nc = _STATE["nc"]
    in_maps = _prepack(current_pose, w_current, w_next, E_proj, rel_embedd)
    trace = os.environ.get("KERNEL_TRACE") == "1"
    res = bass_utils.run_bass_kernel_spmd(
        nc, in_maps, core_ids=list(range(NCORES)), trace=trace
    )
    _STATE["last_result"] = res
    acc = np.zeros((128, 1024), dtype=np.float32)
    for c in range(NCORES):
        acc += np.asarray(res.results[c]["out"], dtype=np.float32)
    # [(i4, b), (h, o, j)] -> (b, o, h*4+i4, j)
    out = (
        acc.reshape(4, 32, 2, 64, 8)
        .transpose(1, 3, 2, 0, 4)
        .reshape(B, OUT_N, POSE)
    )
    return np.ascontiguousarray(out[:, None, :, :])
